# revision 55
# baseline (speedup 1.0000x reference)
"""MoE routing transformer block on 8 trn2 NeuronCores.

Strategy: the reference's (top-k slot kk, expert e) pairs partition the
T=2048 tokens into 8 independent groups (2 slots x 4 experts), each running a
full pre-LN attention+MLP block with attention restricted to the group.
One NeuronCore per (kk, e) pair.

Host: computes the (tiny) router gate + top-2 routing in numpy, gathers each
group's tokens, pre-transposes/packs weights, launches one SPMD bass kernel
on the 8 cores, then scatter-adds the gate-prob-weighted outputs back.

Device (per core, everything in transposed [feature, token] layout):
  hT = LN1(xT)                        computed on host, shipped bf16
  qkT = WqkT.T @ hT (+bias)           q pre-scaled by 1/sqrt(D) on host
  v   = hT.T @ WvT (+bias row)        normal layout, per-head 65-col groups
                                      with a ones column for the denominator
  sT[k,q] = kT_h.T @ qT_h             per (head, k-tile)
  expT = exp(sT + key_bias)           key_bias kills padded keys
  oT'[d,q], den[q] = v_aug.T @ expT   accumulated over k-tiles
  oT = oT' * bcast(exp(-ln(den)))     table-based reciprocal, same act set
  aoT = WoT.T @ oT ; x1T = xT + aoT + bo
  h2T = LN2(x1T)
  gT = gelu(W1T.T @ h2T + b1)         gT kept in bf16
  yT = x1T + W2T.T @ gT + b2          w2 fully resident in SBUF
Matmuls run as float32r/bf16. Queries padded to Cq (mult of 64), keys to
Ck (mult of 128); qkT key-tail columns are zeroed so padded keys are
killed by the -60 exp bias (hT is zero-padded to Ck on the host).

Perf-critical structure (why it's shaped this way):
 - few, large, contiguous DMAs (host pre-packs [128, N] buffers);
   weights on the scalar-engine HWDGE ring, activations on the sync ring
 - w1/w2 fully preloaded (no mid-MLP DMA stalls)
 - attention emitted head-PAIR-wise: the two 64-partition score matmuls of
   a pair run concurrently (row-group tiling), all five exps of a head
   pipeline behind the scores, and the next qkT pair is emitted between
   scores and AV so the PE never idles long enough to re-throttle (HAM)
 - softmax denominators: exp(-ln(d)) on the scalar engine -- both functions
   live in the already-loaded natural_log_exp act-table set; slot 0
   (heads 0-3) is processed mid-attention so only slot 1 is on the tail
 - LN2 stats interleaved with the out-projection; pA/pB live in the po
   psum pool so mlp1 matmuls start while LN2 finishes
 - mlp2 for nt 0,1 is interleaved into the mlp1 loop (lag 2) using the po
   psum pool; nt 2,3 follow densely after
 - act-table priming: dummy Ln at t=0; dummy Gelu fed from rstd so the
   gelu table load overlaps the pA/pB + mlp1 matmuls
"""

import os
import numpy as np
import ml_dtypes

import concourse.bass as bass
import concourse.mybir as mybir
import concourse.tile as tile
import concourse.tile_utils as tile_utils
from concourse import bass_utils


def _install_ntff_shim():
    """This image's antenv lacks axon_hooks; synthesize it so trace=True works."""
    import sys as _sys
    import types as _types
    try:
        import antenv.axon_hooks  # noqa: F401
        return
    except ImportError:
        pass
    try:
        from trn_agent_boot.trn_boot import _ntff_profile_via_ctypes
        hook = _ntff_profile_via_ctypes('/opt/axon/libaxon_pjrt.so')
    except Exception:
        hook = None
    mod = _types.ModuleType('antenv.axon_hooks')
    state = {'hook': hook}
    mod.set_axon_ntff_profile_hook = lambda h: state.__setitem__('hook', h)
    mod.get_axon_ntff_profile_hook = lambda: state['hook']
    _sys.modules['antenv.axon_hooks'] = mod
    try:
        import antenv
        antenv.axon_hooks = mod
    except ImportError:
        pass


_install_ntff_shim()

# stale constant leaves 16KiB/partition unused on trn2 (224 phys / 208 usable)
tile_utils.max_sbuf_usage = 208 * 1024

E = 512
H = 8
D = 64
HID = 2048
NE = 4
TOPK = 2
EPS = 1e-5

f32 = mybir.dt.float32
f32r = mybir.dt.float32r
bf16 = mybir.dt.bfloat16
AF = mybir.ActivationFunctionType
ALU = mybir.AluOpType

KEY_PAD_BIAS = -60.0


# ---------------------------------------------------------------------------
# walrus in this container encodes at most one sync wait per instruction;
# Tile's kernel-tail drain can carry several. Split extras onto NoOps.
def _split_excess_waits(nc):
    for fn in nc.m.functions:
        for blk in fn.blocks:
            new_insts = []
            for ins in blk.instructions:
                si = ins.sync_info
                if si is not None and len(si.on_wait) > 1:
                    waits = list(si.on_wait)
                    excess, keep = waits[:-1], waits[-1:]
                    for w in excess:
                        new_insts.append(mybir.InstNoOp(
                            name=f"I-waitsplit-{nc.next_id()}",
                            engine=ins.engine, ins=[], outs=[],
                            sync_info=mybir.SyncInfo(on_wait=[w], on_update=[]),
                        ))
                    si.on_wait = keep
                new_insts.append(ins)
            blk.instructions[:] = new_insts


def _chunks(Cq):
    """Equal moving-dim chunks <= 512 (each a multiple of 32)."""
    if Cq <= 512:
        return [(0, Cq)]
    assert Cq % 64 == 0 and Cq <= 1024
    h = Cq // 2
    return [(0, h), (h, h)]


def _build(Cq, Ck, phases=99):
    """Bass program: Cq = query capacity (mult 64), Ck = key cap (mult 128)."""
    KT = Ck // 128
    CH = _chunks(Cq)
    NCH = len(CH)
    assert NCH <= 2
    nc = bass.Bass(num_swdge_queues=4)

    hT_d = nc.dram_tensor("hT", [128, 4 * Ck], bf16, kind="ExternalInput")
    xgT_d = nc.dram_tensor("xgT", [128, 4 * Cq], f32r, kind="ExternalInput")
    NCONST = KT + 8 + 4 + 16 + 4 + 4 + 1
    consts_d = nc.dram_tensor("consts", [128, NCONST], f32, kind="ExternalInput")
    sel8_d = nc.dram_tensor("sel8", [128, 256], f32r, kind="ExternalInput")
    onesb_d = nc.dram_tensor("onesb", [1, 1536], bf16, kind="ExternalInput")
    wqka_d = nc.dram_tensor("wqka", [128, 1024], bf16, kind="ExternalInput")
    wqkb_d = nc.dram_tensor("wqkb", [128, 3072], bf16, kind="ExternalInput")
    wv_d = nc.dram_tensor("wv", [128, 2048], bf16, kind="ExternalInput")
    wo_d = nc.dram_tensor("wo", [128, 2048], bf16, kind="ExternalInput")
    fp8 = mybir.dt.float8e4
    w1_d = nc.dram_tensor("w1", [128, 8192], fp8, kind="ExternalInput")
    w2_d = nc.dram_tensor("w2", [128, 8192], fp8, kind="ExternalInput")
    out_d = nc.dram_tensor("yT", [128, 4 * Cq], bf16, kind="ExternalOutput")

    with tile.TileContext(nc) as tc, nc.allow_low_precision(
            reason="float32r/bf16 rounding on matmul-feeding tiles is intended"):
        with (
            tc.tile_pool(name="const", bufs=1) as cpool,
            tc.tile_pool(name="main", bufs=1) as mpool,
            tc.tile_pool(name="hpool", bufs=1) as hpool,
            tc.tile_pool(name="sqpool", bufs=1) as sqpool,
            tc.tile_pool(name="scr", bufs=2) as scr,
            tc.tile_pool(name="stat", bufs=1) as stat,
            tc.tile_pool(name="expp", bufs=24) as expp,
        ):
            # ---- tiles ----
            cst = cpool.tile([128, NCONST], f32)
            sel8 = cpool.tile([128, 256], f32r)
            onesb = cpool.tile([1, 1536], bf16)
            hT = hpool.tile([128, 4, Ck], bf16, tag="h")
            xT = mpool.tile([128, 4, Cq], f32r, tag="xT")
            # qk weights in pair-major order: slot 2t = q head-pair t,
            # slot 2t+1 = k head-pair t; each slot is [kt*128+c] feature-major
            wqka = mpool.tile([128, 2, 512], bf16, tag="wqka")
            wqkb = mpool.tile([128, 6, 512], bf16, tag="wqkb")
            wv = mpool.tile([128, 4, 512], bf16, tag="wv")
            wo = mpool.tile([128, 4, 512], bf16, tag="wo")
            # mlp weights in fp8 DoubleRow pair-major blocks: [p, kp, j, m]
            # holds W[(2*kp+j)*128+p, m] * 8 (scaled into fp8 normal range)
            w1 = mpool.tile([128, 2, 2, 2048], fp8, tag="w1")
            w2 = mpool.tile([128, 8, 2, 512], fp8, tag="w2")
            qkT = mpool.tile([128, 8, Ck], bf16, tag="qkT")
            v = mpool.tile([128, KT, 8 * 65], bf16, tag="v")
            onorm = mpool.tile([128, 4, Cq], bf16, tag="onorm")
            x1T = mpool.tile([128, 4, Cq], f32r, tag="x1T")
            gT = mpool.tile([128, 16, Cq], fp8, tag="gT")
            yT = mpool.tile([128, 4, Cq], bf16, tag="yT")
            # head h denominator -> partition 32*(h%4), slot h//4.
            # den is f32r so the rp broadcast matmul runs single-pass.
            den = stat.tile([128, 2, Cq], f32r, tag="den")
            den_ln = stat.tile([128, 2, Cq], f32, tag="den_ln")

            # ---- pure memsets first: keep the vector queue free of
            # cst-DMA-dependent work so eps/dummy-Ln can run immediately ----
            eps_t = cpool.tile([1, 1], f32)
            nc.vector.memset(eps_t[:], EPS)
            ones_rowf = cpool.tile([1, 128], f32)
            nc.vector.memset(ones_rowf[:], 1.0)
            ones_row = cpool.tile([1, 128], f32r)
            nc.vector.tensor_copy(ones_row[:], ones_rowf[:])
            dummy = cpool.tile([1, 2], f32)
            # prime the natural_log_exp act-table set while DMAs run
            nc.scalar.activation(dummy[0:1, 0:1], eps_t[0:1, 0:1], AF.Ln)
            if Ck > Cq:
                nc.vector.memset(qkT[:, :, Cq:Ck], 0.0)
            # unwritten den rows must not be NaN (0 * NaN = NaN in rp matmul);
            # DVE memset can't write f32r, so stage via den_ln (f32) + cast
            nc.vector.memset(den_ln[:], 1.0)
            nc.vector.tensor_copy(den[:], den_ln[:])
            # ones column of v_aug (attention denominator accumulator)
            nc.vector.memset(
                v[:].rearrange("p t (h x) -> p t h x", x=65)[:, :, :, 64:65], 1.0)

            # ---- critical-path DMAs on the scalar HWDGE ring (starts
            # transferring ~2us earlier than the sync ring in practice) ----
            hT_v = hT_d[:].rearrange("p (t c) -> p t c", t=4)
            nc.scalar.dma_start(hT[:, 0:2, :], hT_v[:, 0:2, :])
            nc.scalar.dma_start(hT[:, 2:4, :], hT_v[:, 2:4, :])
            nc.scalar.dma_start(wqka[:],
                                wqka_d[:].rearrange("p (t c) -> p t c", t=2))
            nc.scalar.dma_start(wv[:], wv_d[:].rearrange("p (t c) -> p t c", t=4))
            nc.scalar.dma_start(wqkb[:],
                                wqkb_d[:].rearrange("p (t c) -> p t c", t=6))
            nc.scalar.dma_start(wo[:], wo_d[:].rearrange("p (t c) -> p t c", t=4))
            nc.scalar.dma_start(
                w1[:], w1_d[:].rearrange("p (t j c) -> p t j c", t=2, j=2))
            nc.scalar.dma_start(
                w2[:], w2_d[:].rearrange("p (t j c) -> p t j c", t=8, j=2))
            # ---- the rest on the sync ring ----
            nc.sync.dma_start(cst[:], consts_d[:])
            nc.sync.dma_start(sel8[:], sel8_d[:])
            nc.sync.dma_start(onesb[:], onesb_d[:])
            nc.sync.dma_start(xT[:], xgT_d[:].rearrange("p (t c) -> p t c", t=4))

            # const slices
            o = [0]
            def _csl(n):
                a = o[0]; o[0] += n
                return cst[:, a:a + n]
            kb, bqk, bo, b1, b2c, l2w = (_csl(KT), _csl(8), _csl(4), _csl(16),
                                         _csl(4), _csl(4))
            ones_colf = _csl(1)

            ecol = cpool.tile([128, 1], f32r)
            nc.vector.tensor_copy(ecol[:], ones_colf)
            ecol2 = cpool.tile([128, 1], f32r)
            nc.vector.tensor_scalar_mul(ecol2[:], ecol[:], 1.0 / E)

            def pview(p):
                """[128, 2, 512] psum tile -> strided chunk view matching [*, Cq]."""
                if NCH == 1:
                    return p[:, 0, 0:CH[0][1]]
                return p[:, :, 0:CH[0][1]]

            psA_cm = tc.tile_pool(name="psA", bufs=2, space="PSUM")
            ps = psA_cm.__enter__()
            att_po = tc.tile_pool(name="att_o", bufs=2, space="PSUM")
            ps_o = att_po.__enter__()

            def emit_qk_pair(t):
                """qkT for nt = t (q head pair t) and nt = 4+t (k head pair t)."""
                for j, nt in ((2 * t, t), (2 * t + 1, 4 + t)):
                    wsl = wqka[:, j, :] if j < 2 else wqkb[:, j - 2, :]
                    p = ps.tile([128, 2, 512], f32, tag="b2", name=f"qk{nt}")
                    for kt in range(4):
                        for ci, (off, sz) in enumerate(CH):
                            nc.tensor.matmul(p[:, ci, 0:sz],
                                             wsl[:, 128 * kt:128 * (kt + 1)],
                                             hT[:, kt, off:off + sz],
                                             start=(kt == 0), stop=(kt == 3))
                    nc.vector.tensor_scalar_add(qkT[:, nt, 0:Cq], pview(p),
                                                bqk[:, nt:nt + 1])

            def emit_v():
                for tt in range(KT):
                    p = ps.tile([128, 2, 512], f32, tag="b2", name=f"v{tt}")
                    for kt in range(4):
                        nc.tensor.matmul(p[:, 0, :],
                                         hT[:, kt, 128 * tt:128 * (tt + 1)],
                                         wv[:, kt, :], start=(kt == 0), stop=False)
                    nc.tensor.matmul(p[:, 0, :], onesb[0:1, 0:128],
                                     onesb[0:1, 512:1024], start=False, stop=True)
                    nc.vector.tensor_copy(
                        v[:, tt, :].rearrange("p (h x) -> p h x", x=65)[:, :, 0:64],
                        p[:, 0, :].rearrange("p (h x) -> p h x", x=64))

            def den_recip(sl):
                """1/d = exp(-ln(d)) for den slot sl (scalar engine only)."""
                nc.scalar.activation(den_ln[:, sl, :], den[:, sl, :], AF.Ln)
                nc.scalar.activation(den[:, sl, :], den_ln[:, sl, :],
                                     AF.Exp, scale=-1.0)

            def den_apply(t):
                """Broadcast slot t//2's reciprocals and scale onorm pair t."""
                rp = ps.tile([128, 2, 512], f32, tag="b2", name=f"rp{t}")
                for ci, (off, sz) in enumerate(CH):
                    nc.tensor.matmul(rp[:, ci, 0:sz],
                                     sel8[:, 128 * (t % 2):128 * (t % 2 + 1)],
                                     den[:, t // 2, off:off + sz],
                                     start=True, stop=True)
                nc.vector.tensor_mul(onorm[:, t, :], onorm[:, t, :], pview(rp))

            if phases >= 3:
                emit_qk_pair(0)

            # ---- attention, head-pair-wise, AV lagging scores by one pair ----
            if phases >= 5:
                all_ets = {}

                def emit_scores(t):
                    hA, hB = 2 * t, 2 * t + 1
                    ets = {hA: [], hB: []}
                    # the two heads' 64-partition matmuls run concurrently in
                    # the PE array (distinct row groups)
                    for kt in range(KT):
                        for h in (hA, hB):
                            bp = 64 * (h % 2)
                            pss = ps.tile([128, 2, 512], f32, tag="b2",
                                          name=f"s{h}_{kt}")
                            for ci, (off, sz) in enumerate(CH):
                                nc.tensor.matmul(
                                    pss[:, ci, 0:sz],
                                    qkT[bp:bp + 64, 4 + t, 128 * kt:128 * (kt + 1)],
                                    qkT[bp:bp + 64, t, off:off + sz],
                                    start=True, stop=True)
                            et = expp.tile([128, Cq], bf16, tag="expT",
                                           name=f"et{h}_{kt}")
                            nc.scalar.activation(et[:, :], pview(pss),
                                                 AF.Exp, bias=kb[:, kt:kt + 1])
                            ets[h].append(et)
                    all_ets[t] = ets

                def emit_av(t):
                    hA, hB = 2 * t, 2 * t + 1
                    ets = all_ets.pop(t)
                    po = {h: ps_o.tile([65, 2, 512], f32, tag="po", name=f"po{h}")
                          for h in (hA, hB)}
                    for kt in range(KT):
                        for h in (hA, hB):
                            for ci, (off, sz) in enumerate(CH):
                                nc.tensor.matmul(po[h][0:65, ci, 0:sz],
                                                 v[:, kt, 65 * h:65 * h + 65],
                                                 ets[h][kt][:, off:off + sz],
                                                 start=(kt == 0),
                                                 stop=(kt == KT - 1))
                    for h in (hA, hB):
                        nc.vector.tensor_copy(den[32 * (h % 4):32 * (h % 4) + 1,
                                                  h // 4, :],
                                              pview(po[h][0:65])[64:65])
                    for h in (hA, hB):
                        nc.vector.tensor_copy(
                            onorm[64 * (h % 2):64 * (h % 2) + 64, t, :],
                            pview(po[h][0:65])[0:64])

                if phases >= 4:
                    emit_v()
                emit_scores(0)
                for t in range(4):
                    if t < 3:
                        if phases >= 3:
                            emit_qk_pair(t + 1)
                        emit_scores(t + 1)
                    emit_av(t)
                    if t == 1:
                        den_recip(0)    # heads 0-3 done: hide slot 0 here
                den_recip(1)
                for t in range(4):
                    den_apply(t)

            # ---- out proj + residual + interleaved LN2 stats ----
            if phases >= 6:
                sq = sqpool.tile([128, 4, Cq], f32r, tag="sq")
                stm = ps_o.tile([1, 2, 512], f32, tag="po", name="ln_stm")
                stq = ps_o.tile([1, 2, 512], f32, tag="po", name="ln_stq")
                for nt in range(4):
                    p = ps.tile([128, 2, 512], f32, tag="b2", name=f"op{nt}")
                    for ht in range(4):      # head pair (2*ht, 2*ht+1)
                        for ci, (off, sz) in enumerate(CH):
                            nc.tensor.matmul(
                                p[:, ci, 0:sz],
                                wo[:, ht, 128 * nt:128 * (nt + 1)],
                                onorm[:, ht, off:off + sz],
                                start=(ht == 0), stop=(ht == 3))
                    nc.vector.scalar_tensor_tensor(
                        x1T[:, nt, :], pview(p), bo[:, nt:nt + 1],
                        xT[:, nt, :], op0=ALU.add, op1=ALU.add)
                    nc.scalar.activation(sq[:, nt, :], x1T[:, nt, :], AF.Square)
                    if phases >= 7:
                        for ci, (off, sz) in enumerate(CH):
                            nc.tensor.matmul(stm[0:1, ci, 0:sz], ecol2[:],
                                             x1T[:, nt, off:off + sz],
                                             start=(nt == 0), stop=(nt == 3))
                            nc.tensor.matmul(stq[0:1, ci, 0:sz], ecol2[:],
                                             sq[:, nt, off:off + sz],
                                             start=(nt == 0), stop=(nt == 3))

            # ---- LN2 tail ----
            if phases >= 7:
                # h2/8 in fp8 (DoubleRow consumes kt-pair block views)
                h2i = sqpool.tile([128, 4, Cq], fp8, tag="h2i")
                mu2 = stat.tile([1, Cq], f32, tag="mu2")
                nc.scalar.activation(mu2[0:1, :], pview(stm)[0:1], AF.Square)
                var = stat.tile([1, Cq], f32, tag="var")
                nc.vector.scalar_tensor_tensor(
                    var[0:1, :], mu2[0:1, :], -1.0, pview(stq)[0:1],
                    op0=ALU.mult, op1=ALU.add)
                # rstd = exp(-0.5 * ln(var + eps)); Ln+Exp share one ACT table set
                lnv = stat.tile([1, Cq], f32, tag="lnv")
                nc.scalar.activation(lnv[:], var[:], AF.Ln, bias=eps_t[0:1, 0:1])
                rstd = stat.tile([1, Cq], f32r, tag="rstd")
                nc.scalar.activation(rstd[:], lnv[:], AF.Exp, scale=-0.5)
                # prime the gelu table set; depends on rstd so it cannot be
                # hoisted earlier than this point by the scheduler
                nc.scalar.activation(dummy[0:1, 1:2], rstd[0:1, 0:1], AF.Gelu)
                mbneg = stat.tile([1, Cq], f32r, tag="mbneg")
                nc.vector.scalar_tensor_tensor(mbneg[:], pview(stm)[0:1],
                                               -1.0, rstd[:],
                                               op0=ALU.mult, op1=ALU.mult)
                pA = ps_o.tile([128, 2, 512], f32, tag="po", name="ln_A")
                pB = ps_o.tile([128, 2, 512], f32, tag="po", name="ln_B")
                for ci, (off, sz) in enumerate(CH):
                    nc.tensor.matmul(pA[:, ci, 0:sz], ones_row[0:1, :],
                                     rstd[0:1, off:off + sz], start=True, stop=True)
                    nc.tensor.matmul(pB[:, ci, 0:sz], ones_row[0:1, :],
                                     mbneg[0:1, off:off + sz], start=True, stop=True)
                for kt in range(4):
                    tmp = scr.tile([128, Cq], f32, tag="lntmp")
                    nc.vector.scalar_tensor_tensor(
                        tmp[:], x1T[:, kt, :], l2w[:, kt:kt + 1], pview(pA),
                        op0=ALU.mult, op1=ALU.mult)
                    nc.vector.scalar_tensor_tensor(
                        h2i[:, kt, :], pview(pB),
                        l2w[:, kt:kt + 1], tmp[:],
                        op0=ALU.mult, op1=ALU.add)

            # ---- mlp1 (+ mlp2 for nt 0,1 interleaved at lag 2) ----
            if phases >= 8:
                pm = {}

                def mlp2_emit(kp, nts):
                    for nt in nts:
                        for ci, (off, sz) in enumerate(CH):
                            nc.tensor.matmul(
                                pm[nt][:, ci, 0:sz],
                                w2[:, kp, :, 128 * nt:128 * (nt + 1)],
                                gT[:, 2 * kp:2 * kp + 2, off:off + sz],
                                perf_mode=mybir.MatmulPerfMode.DoubleRow,
                                start=(kp == 0), stop=False)
                    if kp == 7:     # + b2*8 per output feature, closes group
                        for nt in nts:
                            for ci, (off, sz) in enumerate(CH):
                                nc.tensor.matmul(
                                    pm[nt][:, ci, 0:sz],
                                    onesb[0:1, 1024 + 128 * nt:1152 + 128 * nt],
                                    onesb[0:1, off:off + sz],
                                    start=False, stop=True)

                def mlp2_fin(nt):
                    for ci, (off, sz) in enumerate(CH):
                        nc.vector.scalar_tensor_tensor(
                            yT[:, nt, off:off + sz], pm[nt][:, ci, 0:sz],
                            0.125, x1T[:, nt, off:off + sz],
                            op0=ALU.mult, op1=ALU.add)
                    nc.sync.dma_start(out_d[:, nt * Cq:(nt + 1) * Cq],
                                      yT[:, nt, :])

                for nt in range(16):
                    p = ps.tile([128, 2, 512], f32, tag="b2", name=f"m1{nt}")
                    for kp in range(2):
                        for ci, (off, sz) in enumerate(CH):
                            nc.tensor.matmul(
                                p[:, ci, 0:sz],
                                w1[:, kp, :, 128 * nt:128 * (nt + 1)],
                                h2i[:, 2 * kp:2 * kp + 2, off:off + sz],
                                perf_mode=mybir.MatmulPerfMode.DoubleRow,
                                start=(kp == 0), stop=(kp == 1))
                    nc.scalar.activation(gT[:, nt, :], pview(p),
                                         AF.Gelu, bias=b1[:, nt:nt + 1])
                    if phases >= 9:
                        if nt == 1:
                            pm[0] = ps_o.tile([128, 2, 512], f32, tag="po",
                                              name="pm0")
                            pm[1] = ps_o.tile([128, 2, 512], f32, tag="po",
                                              name="pm1")
                        if nt >= 3 and nt % 2 == 1:
                            mlp2_emit((nt - 3) // 2, (0, 1))
                if phases >= 9:
                    mlp2_emit(7, (0, 1))
                    mlp2_fin(0)
                    mlp2_fin(1)
                    pm[2] = ps.tile([128, 2, 512], f32, tag="b2", name="pm2")
                    pm[3] = ps.tile([128, 2, 512], f32, tag="b2", name="pm3")
                    for kp in range(8):
                        mlp2_emit(kp, (2, 3))
                    mlp2_fin(2)
                    mlp2_fin(3)

            if phases < 9:
                for nt in range(4):
                    nc.vector.tensor_copy(yT[:, nt, :], xT[:, nt, :])
                    nc.sync.dma_start(out_d[:, nt * Cq:(nt + 1) * Cq],
                                      yT[:, nt, :])

            att_po.__exit__(None, None, None)
            psA_cm.__exit__(None, None, None)

    _split_excess_waits(nc)
    return nc


_prog_cache = {}


def _get_prog(Cq, Ck):
    key = (Cq, Ck)
    if key not in _prog_cache:
        _prog_cache[key] = _build(Cq, Ck)
    return _prog_cache[key]


def _route(xf, gate_w, gate_b):
    """Replicate reference routing: top-2 of xf @ gate_w.T + gate_b."""
    logits = xf @ gate_w.T + gate_b            # [T, NE] fp32
    n = len(logits)
    idx0 = np.argmax(logits, axis=1)
    v0 = logits[np.arange(n), idx0]
    masked = logits.copy()
    masked[np.arange(n), idx0] = -np.inf
    idx1 = np.argmax(masked, axis=1)
    v1 = masked[np.arange(n), idx1]
    m = np.maximum(v0, v1)
    e0 = np.exp(v0 - m)
    e1 = np.exp(v1 - m)
    p0 = e0 / (e0 + e1)
    p1 = e1 / (e0 + e1)
    return np.stack([idx0, idx1], 1), np.stack([p0, p1], 1).astype(np.float32)


def _feat_pack(a, tiles, dtype):
    """[tiles*128, N] -> [128, tiles*N] with [p, t*N+c] = a[t*128+p, c]."""
    n = a.shape[1]
    return np.ascontiguousarray(
        a.reshape(tiles, 128, n).transpose(1, 0, 2).reshape(128, tiles * n)
        .astype(dtype))


def kernel(x, gate_w, gate_b, ln1_w, ln1_b, in_proj_w, in_proj_b, out_proj_w,
           out_proj_b, ln2_w, ln2_b, mlp_w1, mlp_b1, mlp_w2, mlp_b2):
    x = np.asarray(x, np.float32)
    B, N, _ = x.shape
    T = B * N
    xf = np.ascontiguousarray(x.reshape(T, E))

    topk_idx, probs = _route(xf, np.asarray(gate_w, np.float32),
                             np.asarray(gate_b, np.float32))

    groups = []          # (token_indices, prob_slice) per core, kk-major
    for kk in range(TOPK):
        for e in range(NE):
            sel = np.nonzero(topk_idx[:, kk] == e)[0]
            groups.append((sel, probs[sel, kk]))
    Cmax = max((len(s) for s, _ in groups), default=128)
    Cq = max(128, -(-Cmax // 64) * 64)
    Ck = max(128, -(-Cmax // 128) * 128)
    KT = Ck // 128

    bfd = ml_dtypes.bfloat16
    ew = []
    for e in range(NE):
        Wq = np.asarray(in_proj_w[e][0:E], np.float32)
        Wk = np.asarray(in_proj_w[e][E:2 * E], np.float32)
        Wv = np.asarray(in_proj_w[e][2 * E:3 * E], np.float32)
        bq = np.asarray(in_proj_b[e][0:E], np.float32)
        bk = np.asarray(in_proj_b[e][E:2 * E], np.float32)
        bv = np.asarray(in_proj_b[e][2 * E:3 * E], np.float32)
        l1b = np.asarray(ln1_b[e], np.float32)
        l2b = np.asarray(ln2_b[e], np.float32)
        scale = np.float32(1.0) / np.sqrt(np.float32(D))
        wqk = np.concatenate([Wq.T * scale, Wk.T], axis=1)          # [E, 2E]
        bqk = np.concatenate([(Wq @ l1b + bq) * scale, Wk @ l1b + bk])
        # pair-major slots: j=2t -> q pair t (nt=t), j=2t+1 -> k pair t (nt=4+t)
        arr = wqk.reshape(4, 128, 8, 128)          # [kt, p, nt, c]
        pm = arr[:, :, [0, 4, 1, 5, 2, 6, 3, 7], :]
        pm = pm.transpose(1, 2, 0, 3).reshape(128, 8, 512)
        w1 = np.asarray(mlp_w1[e], np.float32)
        fp8d = ml_dtypes.float8_e4m3
        # DoubleRow pair-major [p, kp, j, m] = W.T[(2kp+j)*128+p, m] * 8
        w1t8 = (w1.T * 8.0).reshape(2, 2, 128, HID).transpose(2, 0, 1, 3)
        w2t8 = (np.asarray(mlp_w2[e], np.float32).T * 8.0)\
            .reshape(8, 2, 128, E).transpose(2, 0, 1, 3)
        ew.append(dict(
            wqka=np.ascontiguousarray(pm[:, 0:2, :].reshape(128, 1024)
                                      .astype(bfd)),
            wqkb=np.ascontiguousarray(pm[:, 2:8, :].reshape(128, 3072)
                                      .astype(bfd)),
            bqk=np.ascontiguousarray(bqk, np.float32),
            wv=_feat_pack(Wv.T, 4, bfd),
            bvr=np.ascontiguousarray(Wv @ l1b + bv, np.float32),
            wo=_feat_pack(np.asarray(out_proj_w[e], np.float32).T, 4, bfd),
            bo=np.ascontiguousarray(out_proj_b[e], np.float32),
            w1=np.ascontiguousarray(w1t8.reshape(128, 8192).astype(fp8d)),
            b1=np.ascontiguousarray(w1 @ l2b + np.asarray(mlp_b1[e], np.float32)),
            w2=np.ascontiguousarray(w2t8.reshape(128, 8192).astype(fp8d)),
            b2=np.ascontiguousarray(mlp_b2[e], np.float32),
            l1w=np.ascontiguousarray(ln1_w[e], np.float32),
            l2w=np.ascontiguousarray(ln2_w[e], np.float32),
        ))

    def colpack(vec, ncol):
        a = np.zeros((128, ncol), np.float32)
        v = np.asarray(vec, np.float32).reshape(-1)
        a[:, :] = v.reshape(ncol, 128).T
        return a

    # rp broadcast selector: head h's reciprocal denominator lives on den
    # partition 32*(h%4), slot h//4; rp[t] (head pair 2t,2t+1) maps it to
    # output partitions 0:64 / 64:128. Block A: pairs on partitions {0,32},
    # block B: pairs on {64,96}.
    sel8_np = np.zeros((128, 256), np.float32)
    sel8_np[0, 0:64] = 1.0
    sel8_np[32, 64:128] = 1.0
    sel8_np[64, 128:192] = 1.0
    sel8_np[96, 192:256] = 1.0

    in_maps = []
    for ci, (sel, _p) in enumerate(groups):
        e = ci % NE
        w = ew[e]
        S = len(sel)
        xg = xf[sel]
        xgT = np.zeros((E, Cq), np.float32)
        xgT[:, :S] = xg.T
        mu_h = xg.mean(1, keepdims=True)
        var_h = ((xg - mu_h) ** 2).mean(1, keepdims=True)
        hg = (xg - mu_h) / np.sqrt(var_h + EPS) * w["l1w"][None, :]
        hT_np = np.zeros((E, Ck), np.float32)
        hT_np[:, :S] = hg.T
        kbias = np.full((Ck,), KEY_PAD_BIAS, np.float32)
        kbias[:S] = 0.0
        consts = np.concatenate([
            colpack(kbias, KT), colpack(w["bqk"], 8), colpack(w["bo"], 4),
            colpack(w["b1"], 16), colpack(w["b2"], 4),
            colpack(w["l2w"] / 8.0, 4), np.ones((128, 1), np.float32)], axis=1)
        onesb_np = np.zeros((1, 1536), np.float32)
        onesb_np[0, 0:512] = 1.0
        onesb_np[0, 512:1024] = w["bvr"]
        onesb_np[0, 1024:1536] = w["b2"] * 8.0
        in_maps.append({
            "hT": _feat_pack(hT_np, 4, bfd),
            "xgT": _feat_pack(xgT, 4, np.float32),
            "consts": consts,
            "sel8": sel8_np,
            "onesb": onesb_np.astype(bfd),
            "wqka": w["wqka"], "wqkb": w["wqkb"], "wv": w["wv"], "wo": w["wo"],
            "w1": w["w1"], "w2": w["w2"],
        })

    nc = _get_prog(Cq, Ck)
    res = bass_utils.run_bass_kernel_spmd(
        nc, in_maps, core_ids=list(range(8)),
        trace=bool(int(os.environ.get("KERNEL_TRACE", "0"))))
    kernel.last_exec_time_ns = res.exec_time_ns
    kernel.last_results = res

    out = np.zeros((T, E), np.float32)
    for ci, (sel, p) in enumerate(groups):
        S = len(sel)
        if S == 0:
            continue
        yT = np.asarray(res.results[ci]["yT"], np.float32)     # [128, 4*Cq]
        yTf = yT.reshape(128, 4, Cq).transpose(1, 0, 2).reshape(E, Cq)
        out[sel] += yTf[:, :S].T * p[:, None]
    return out.reshape(B, N, E)


# revision 56
# speedup vs baseline: 1.0987x; 1.0987x over previous
"""MoE routing transformer block on 8 trn2 NeuronCores.

Strategy: the reference's (top-k slot kk, expert e) pairs partition the
T=2048 tokens into 8 independent groups (2 slots x 4 experts), each running a
full pre-LN attention+MLP block with attention restricted to the group.
One NeuronCore per (kk, e) pair.

Host: computes the (tiny) router gate + top-2 routing in numpy, gathers each
group's tokens, pre-transposes/packs weights, launches one SPMD bass kernel
on the 8 cores, then scatter-adds the gate-prob-weighted outputs back.

Device (per core, everything in transposed [feature, token] layout):
  hT = LN1(xT)                        computed on host, shipped bf16
  qkT = WqkT.T @ hT (+bias)           q pre-scaled by 1/sqrt(D) on host
  v   = hT.T @ WvT (+bias row)        normal layout, per-head 65-col groups
                                      with a ones column for the denominator
  sT[k,q] = kT_h.T @ qT_h             per (head, k-tile)
  expT = exp(sT + key_bias)           key_bias kills padded keys
  oT'[d,q], den[q] = v_aug.T @ expT   accumulated over k-tiles
  oT = oT' * bcast(exp(-ln(den)))     table-based reciprocal, same act set
  aoT = WoT.T @ oT ; x1T = xT + aoT + bo
  h2T = LN2(x1T)
  gT = gelu(W1T.T @ h2T + b1)         gT kept in bf16
  yT = x1T + W2T.T @ gT + b2          w2 fully resident in SBUF
Matmuls run as float32r/bf16. Queries padded to Cq (mult of 64), keys to
Ck (mult of 128); qkT key-tail columns are zeroed so padded keys are
killed by the -60 exp bias (hT is zero-padded to Ck on the host).

Perf-critical structure (why it's shaped this way):
 - few, large, contiguous DMAs (host pre-packs [128, N] buffers);
   weights on the scalar-engine HWDGE ring, activations on the sync ring
 - w1/w2 fully preloaded (no mid-MLP DMA stalls)
 - attention emitted head-PAIR-wise: the two 64-partition score matmuls of
   a pair run concurrently (row-group tiling), all five exps of a head
   pipeline behind the scores, and the next qkT pair is emitted between
   scores and AV so the PE never idles long enough to re-throttle (HAM)
 - softmax denominators: exp(-ln(d)) on the scalar engine -- both functions
   live in the already-loaded natural_log_exp act-table set; slot 0
   (heads 0-3) is processed mid-attention so only slot 1 is on the tail
 - LN2 stats interleaved with the out-projection; pA/pB live in the po
   psum pool so mlp1 matmuls start while LN2 finishes
 - mlp2 for nt 0,1 is interleaved into the mlp1 loop (lag 2) using the po
   psum pool; nt 2,3 follow densely after
 - act-table priming: dummy Ln at t=0; dummy Gelu fed from rstd so the
   gelu table load overlaps the pA/pB + mlp1 matmuls
"""

import os
import numpy as np
import ml_dtypes

import concourse.bass as bass
import concourse.mybir as mybir
import concourse.tile as tile
import concourse.tile_utils as tile_utils
from concourse import bass_utils


def _install_ntff_shim():
    """This image's antenv lacks axon_hooks; synthesize it so trace=True works."""
    import sys as _sys
    import types as _types
    try:
        import antenv.axon_hooks  # noqa: F401
        return
    except ImportError:
        pass
    try:
        from trn_agent_boot.trn_boot import _ntff_profile_via_ctypes
        hook = _ntff_profile_via_ctypes('/opt/axon/libaxon_pjrt.so')
    except Exception:
        hook = None
    mod = _types.ModuleType('antenv.axon_hooks')
    state = {'hook': hook}
    mod.set_axon_ntff_profile_hook = lambda h: state.__setitem__('hook', h)
    mod.get_axon_ntff_profile_hook = lambda: state['hook']
    _sys.modules['antenv.axon_hooks'] = mod
    try:
        import antenv
        antenv.axon_hooks = mod
    except ImportError:
        pass


_install_ntff_shim()

# stale constant leaves 16KiB/partition unused on trn2 (224 phys / 208 usable)
tile_utils.max_sbuf_usage = 208 * 1024

E = 512
H = 8
D = 64
HID = 2048
NE = 4
TOPK = 2
EPS = 1e-5

f32 = mybir.dt.float32
f32r = mybir.dt.float32r
bf16 = mybir.dt.bfloat16
AF = mybir.ActivationFunctionType
ALU = mybir.AluOpType

KEY_PAD_BIAS = -60.0


# ---------------------------------------------------------------------------
# walrus in this container encodes at most one sync wait per instruction;
# Tile's kernel-tail drain can carry several. Split extras onto NoOps.
def _split_excess_waits(nc):
    for fn in nc.m.functions:
        for blk in fn.blocks:
            new_insts = []
            for ins in blk.instructions:
                si = ins.sync_info
                if si is not None and len(si.on_wait) > 1:
                    waits = list(si.on_wait)
                    excess, keep = waits[:-1], waits[-1:]
                    for w in excess:
                        new_insts.append(mybir.InstNoOp(
                            name=f"I-waitsplit-{nc.next_id()}",
                            engine=ins.engine, ins=[], outs=[],
                            sync_info=mybir.SyncInfo(on_wait=[w], on_update=[]),
                        ))
                    si.on_wait = keep
                new_insts.append(ins)
            blk.instructions[:] = new_insts


def _chunks(Cq):
    """Equal moving-dim chunks <= 512 (each a multiple of 32)."""
    if Cq <= 512:
        return [(0, Cq)]
    assert Cq % 64 == 0 and Cq <= 1024
    h = Cq // 2
    return [(0, h), (h, h)]


def _build(Cq, Ck, phases=99):
    """Bass program: Cq = query capacity (mult 64), Ck = key cap (mult 128)."""
    KT = Ck // 128
    CH = _chunks(Cq)
    NCH = len(CH)
    assert NCH <= 2
    nc = bass.Bass(num_swdge_queues=4)

    hT_d = nc.dram_tensor("hT", [128, 4 * Ck], bf16, kind="ExternalInput")
    xgT_d = nc.dram_tensor("xgT", [128, 4 * Cq], f32r, kind="ExternalInput")
    NCONST = KT + 8 + 4 + 16 + 4 + 4 + 1
    consts_d = nc.dram_tensor("consts", [128, NCONST], f32, kind="ExternalInput")
    sel8_d = nc.dram_tensor("sel8", [128, 256], f32r, kind="ExternalInput")
    onesb_d = nc.dram_tensor("onesb", [1, 1536], bf16, kind="ExternalInput")
    wqka_d = nc.dram_tensor("wqka", [128, 1024], bf16, kind="ExternalInput")
    wqkb_d = nc.dram_tensor("wqkb", [128, 3072], bf16, kind="ExternalInput")
    wv_d = nc.dram_tensor("wv", [128, 2048], bf16, kind="ExternalInput")
    wo_d = nc.dram_tensor("wo", [128, 2048], bf16, kind="ExternalInput")
    fp8 = mybir.dt.float8e4
    w1_d = nc.dram_tensor("w1", [128, 8192], fp8, kind="ExternalInput")
    w2_d = nc.dram_tensor("w2", [128, 8192], fp8, kind="ExternalInput")
    out_d = nc.dram_tensor("yT", [128, 4 * Cq], bf16, kind="ExternalOutput")

    with tile.TileContext(nc) as tc, nc.allow_low_precision(
            reason="float32r/bf16 rounding on matmul-feeding tiles is intended"):
        with (
            tc.tile_pool(name="const", bufs=1) as cpool,
            tc.tile_pool(name="main", bufs=1) as mpool,
            tc.tile_pool(name="hpool", bufs=1) as hpool,
            tc.tile_pool(name="sqpool", bufs=1) as sqpool,
            tc.tile_pool(name="scr", bufs=2) as scr,
            tc.tile_pool(name="stat", bufs=1) as stat,
            tc.tile_pool(name="expp", bufs=24) as expp,
        ):
            # ---- tiles ----
            cst = cpool.tile([128, NCONST], f32)
            sel8 = cpool.tile([128, 256], f32r)
            onesb = cpool.tile([1, 1536], bf16)
            hT = hpool.tile([128, 4, Ck], bf16, tag="h")
            xT = mpool.tile([128, 4, Cq], f32r, tag="xT")
            # qk weights in pair-major order: slot 2t = q head-pair t,
            # slot 2t+1 = k head-pair t; each slot is [kt*128+c] feature-major
            wqka = mpool.tile([128, 2, 512], bf16, tag="wqka")
            wqkb = mpool.tile([128, 6, 512], bf16, tag="wqkb")
            wv = mpool.tile([128, 4, 512], bf16, tag="wv")
            wo = mpool.tile([128, 4, 512], bf16, tag="wo")
            # mlp weights in fp8 DoubleRow pair-major blocks: [p, kp, j, m]
            # holds W[(2*kp+j)*128+p, m] * 8 (scaled into fp8 normal range)
            w1 = mpool.tile([128, 2, 2, 2048], fp8, tag="w1")
            w2 = mpool.tile([128, 8, 2, 512], fp8, tag="w2")
            qkT = mpool.tile([128, 8, Ck], bf16, tag="qkT")
            v = mpool.tile([128, KT, 8 * 65], bf16, tag="v")
            onorm = mpool.tile([128, 4, Cq], bf16, tag="onorm")
            x1T = mpool.tile([128, 4, Cq], f32r, tag="x1T")
            gT = mpool.tile([128, 16, Cq], fp8, tag="gT")
            yT = mpool.tile([128, 4, Cq], bf16, tag="yT")
            # head h denominator -> partition 32*(h%4), slot h//4.
            # den is f32r so the rp broadcast matmul runs single-pass.
            den = stat.tile([128, 2, Cq], f32r, tag="den")
            den_ln = stat.tile([128, 2, Cq], f32, tag="den_ln")

            # ---- pure memsets first: keep the vector queue free of
            # cst-DMA-dependent work so eps/dummy-Ln can run immediately ----
            eps_t = cpool.tile([1, 1], f32)
            nc.vector.memset(eps_t[:], EPS)
            ones_rowf = cpool.tile([1, 128], f32)
            nc.vector.memset(ones_rowf[:], 1.0)
            ones_row = cpool.tile([1, 128], f32r)
            nc.vector.tensor_copy(ones_row[:], ones_rowf[:])
            dummy = cpool.tile([1, 2], f32)
            # prime the natural_log_exp act-table set while DMAs run
            nc.scalar.activation(dummy[0:1, 0:1], eps_t[0:1, 0:1], AF.Ln)
            if Ck > Cq:
                nc.vector.memset(qkT[:, :, Cq:Ck], 0.0)
            # unwritten den rows must not be NaN (0 * NaN = NaN in rp matmul);
            # DVE memset can't write f32r, so stage via den_ln (f32) + cast
            nc.vector.memset(den_ln[:], 1.0)
            nc.vector.tensor_copy(den[:], den_ln[:])
            # ones column of v_aug (attention denominator accumulator)
            nc.vector.memset(
                v[:].rearrange("p t (h x) -> p t h x", x=65)[:, :, :, 64:65], 1.0)

            # ---- critical-path DMAs on the scalar HWDGE ring (starts
            # transferring ~2us earlier than the sync ring in practice) ----
            nc.scalar.dma_start(hT[:], hT_d[:].rearrange("p (t c) -> p t c", t=4))
            nc.scalar.dma_start(wqka[:],
                                wqka_d[:].rearrange("p (t c) -> p t c", t=2))
            nc.scalar.dma_start(wv[:], wv_d[:].rearrange("p (t c) -> p t c", t=4))
            nc.scalar.dma_start(wqkb[:],
                                wqkb_d[:].rearrange("p (t c) -> p t c", t=6))
            nc.scalar.dma_start(wo[:], wo_d[:].rearrange("p (t c) -> p t c", t=4))
            nc.scalar.dma_start(
                w1[:], w1_d[:].rearrange("p (t j c) -> p t j c", t=2, j=2))
            nc.scalar.dma_start(
                w2[:], w2_d[:].rearrange("p (t j c) -> p t j c", t=8, j=2))
            # ---- the rest on the sync ring ----
            nc.sync.dma_start(cst[:], consts_d[:])
            nc.sync.dma_start(sel8[:], sel8_d[:])
            nc.sync.dma_start(onesb[:], onesb_d[:])
            nc.sync.dma_start(xT[:], xgT_d[:].rearrange("p (t c) -> p t c", t=4))

            # const slices
            o = [0]
            def _csl(n):
                a = o[0]; o[0] += n
                return cst[:, a:a + n]
            kb, bqk, bo, b1, b2c, l2w = (_csl(KT), _csl(8), _csl(4), _csl(16),
                                         _csl(4), _csl(4))
            ones_colf = _csl(1)

            ecol = cpool.tile([128, 1], f32r)
            nc.vector.tensor_copy(ecol[:], ones_colf)
            ecol2 = cpool.tile([128, 1], f32r)
            nc.vector.tensor_scalar_mul(ecol2[:], ecol[:], 1.0 / E)

            def pview(p):
                """[128, 2, 512] psum tile -> strided chunk view matching [*, Cq]."""
                if NCH == 1:
                    return p[:, 0, 0:CH[0][1]]
                return p[:, :, 0:CH[0][1]]

            psA_cm = tc.tile_pool(name="psA", bufs=2, space="PSUM")
            ps = psA_cm.__enter__()
            att_po = tc.tile_pool(name="att_o", bufs=2, space="PSUM")
            ps_o = att_po.__enter__()

            def emit_qk_pair(t):
                """qkT for nt = t (q head pair t) and nt = 4+t (k head pair t)."""
                for j, nt in ((2 * t, t), (2 * t + 1, 4 + t)):
                    wsl = wqka[:, j, :] if j < 2 else wqkb[:, j - 2, :]
                    p = ps.tile([128, 2, 512], f32, tag="b2", name=f"qk{nt}")
                    for kt in range(4):
                        for ci, (off, sz) in enumerate(CH):
                            nc.tensor.matmul(p[:, ci, 0:sz],
                                             wsl[:, 128 * kt:128 * (kt + 1)],
                                             hT[:, kt, off:off + sz],
                                             start=(kt == 0), stop=(kt == 3))
                    nc.vector.tensor_scalar_add(qkT[:, nt, 0:Cq], pview(p),
                                                bqk[:, nt:nt + 1])

            def emit_v():
                for tt in range(KT):
                    p = ps.tile([128, 2, 512], f32, tag="b2", name=f"v{tt}")
                    for kt in range(4):
                        nc.tensor.matmul(p[:, 0, :],
                                         hT[:, kt, 128 * tt:128 * (tt + 1)],
                                         wv[:, kt, :], start=(kt == 0), stop=False)
                    nc.tensor.matmul(p[:, 0, :], onesb[0:1, 0:128],
                                     onesb[0:1, 512:1024], start=False, stop=True)
                    nc.vector.tensor_copy(
                        v[:, tt, :].rearrange("p (h x) -> p h x", x=65)[:, :, 0:64],
                        p[:, 0, :].rearrange("p (h x) -> p h x", x=64))

            def den_recip(sl):
                """1/d = exp(-ln(d)) for den slot sl (scalar engine only)."""
                nc.scalar.activation(den_ln[:, sl, :], den[:, sl, :], AF.Ln)
                nc.scalar.activation(den[:, sl, :], den_ln[:, sl, :],
                                     AF.Exp, scale=-1.0)

            def den_apply(t):
                """Broadcast slot t//2's reciprocals and scale onorm pair t."""
                rp = ps.tile([128, 2, 512], f32, tag="b2", name=f"rp{t}")
                for ci, (off, sz) in enumerate(CH):
                    nc.tensor.matmul(rp[:, ci, 0:sz],
                                     sel8[:, 128 * (t % 2):128 * (t % 2 + 1)],
                                     den[:, t // 2, off:off + sz],
                                     start=True, stop=True)
                nc.vector.tensor_mul(onorm[:, t, :], onorm[:, t, :], pview(rp))

            if phases >= 3:
                emit_qk_pair(0)

            # ---- attention, head-pair-wise, AV lagging scores by one pair ----
            if phases >= 5:
                all_ets = {}

                def emit_scores(t):
                    hA, hB = 2 * t, 2 * t + 1
                    ets = {hA: [], hB: []}
                    # the two heads' 64-partition matmuls run concurrently in
                    # the PE array (distinct row groups)
                    for kt in range(KT):
                        for h in (hA, hB):
                            bp = 64 * (h % 2)
                            pss = ps.tile([128, 2, 512], f32, tag="b2",
                                          name=f"s{h}_{kt}")
                            for ci, (off, sz) in enumerate(CH):
                                nc.tensor.matmul(
                                    pss[:, ci, 0:sz],
                                    qkT[bp:bp + 64, 4 + t, 128 * kt:128 * (kt + 1)],
                                    qkT[bp:bp + 64, t, off:off + sz],
                                    start=True, stop=True)
                            et = expp.tile([128, Cq], bf16, tag="expT",
                                           name=f"et{h}_{kt}")
                            nc.scalar.activation(et[:, :], pview(pss),
                                                 AF.Exp, bias=kb[:, kt:kt + 1])
                            ets[h].append(et)
                    all_ets[t] = ets

                def emit_av(t):
                    hA, hB = 2 * t, 2 * t + 1
                    ets = all_ets.pop(t)
                    po = {h: ps_o.tile([65, 2, 512], f32, tag="po", name=f"po{h}")
                          for h in (hA, hB)}
                    for kt in range(KT):
                        for h in (hA, hB):
                            for ci, (off, sz) in enumerate(CH):
                                nc.tensor.matmul(po[h][0:65, ci, 0:sz],
                                                 v[:, kt, 65 * h:65 * h + 65],
                                                 ets[h][kt][:, off:off + sz],
                                                 start=(kt == 0),
                                                 stop=(kt == KT - 1))
                    for h in (hA, hB):
                        nc.vector.tensor_copy(den[32 * (h % 4):32 * (h % 4) + 1,
                                                  h // 4, :],
                                              pview(po[h][0:65])[64:65])
                    for h in (hA, hB):
                        nc.vector.tensor_copy(
                            onorm[64 * (h % 2):64 * (h % 2) + 64, t, :],
                            pview(po[h][0:65])[0:64])

                if phases >= 4:
                    emit_v()
                emit_scores(0)
                for t in range(4):
                    if t < 3:
                        if phases >= 3:
                            emit_qk_pair(t + 1)
                        emit_scores(t + 1)
                    emit_av(t)
                    if t == 1:
                        den_recip(0)    # heads 0-3 done: hide slot 0 here
                den_recip(1)
                for t in range(4):
                    den_apply(t)

            # ---- out proj + residual + interleaved LN2 stats ----
            if phases >= 6:
                sq = sqpool.tile([128, 4, Cq], f32r, tag="sq")
                stm = ps_o.tile([1, 2, 512], f32, tag="po", name="ln_stm")
                stq = ps_o.tile([1, 2, 512], f32, tag="po", name="ln_stq")
                for nt in range(4):
                    p = ps.tile([128, 2, 512], f32, tag="b2", name=f"op{nt}")
                    for ht in range(4):      # head pair (2*ht, 2*ht+1)
                        for ci, (off, sz) in enumerate(CH):
                            nc.tensor.matmul(
                                p[:, ci, 0:sz],
                                wo[:, ht, 128 * nt:128 * (nt + 1)],
                                onorm[:, ht, off:off + sz],
                                start=(ht == 0), stop=(ht == 3))
                    nc.vector.scalar_tensor_tensor(
                        x1T[:, nt, :], pview(p), bo[:, nt:nt + 1],
                        xT[:, nt, :], op0=ALU.add, op1=ALU.add)
                    nc.scalar.activation(sq[:, nt, :], x1T[:, nt, :], AF.Square)
                    if phases >= 7:
                        for ci, (off, sz) in enumerate(CH):
                            nc.tensor.matmul(stm[0:1, ci, 0:sz], ecol2[:],
                                             x1T[:, nt, off:off + sz],
                                             start=(nt == 0), stop=(nt == 3))
                            nc.tensor.matmul(stq[0:1, ci, 0:sz], ecol2[:],
                                             sq[:, nt, off:off + sz],
                                             start=(nt == 0), stop=(nt == 3))

            # ---- LN2 tail ----
            if phases >= 7:
                # h2/8 in fp8 (DoubleRow consumes kt-pair block views)
                h2i = sqpool.tile([128, 4, Cq], fp8, tag="h2i")
                mu2 = stat.tile([1, Cq], f32, tag="mu2")
                nc.scalar.activation(mu2[0:1, :], pview(stm)[0:1], AF.Square)
                var = stat.tile([1, Cq], f32, tag="var")
                nc.vector.scalar_tensor_tensor(
                    var[0:1, :], mu2[0:1, :], -1.0, pview(stq)[0:1],
                    op0=ALU.mult, op1=ALU.add)
                # rstd = exp(-0.5 * ln(var + eps)); Ln+Exp share one ACT table set
                lnv = stat.tile([1, Cq], f32, tag="lnv")
                nc.scalar.activation(lnv[:], var[:], AF.Ln, bias=eps_t[0:1, 0:1])
                rstd = stat.tile([1, Cq], f32r, tag="rstd")
                nc.scalar.activation(rstd[:], lnv[:], AF.Exp, scale=-0.5)
                # prime the gelu table set; depends on rstd so it cannot be
                # hoisted earlier than this point by the scheduler
                nc.scalar.activation(dummy[0:1, 1:2], rstd[0:1, 0:1], AF.Gelu)
                mbneg = stat.tile([1, Cq], f32r, tag="mbneg")
                nc.vector.scalar_tensor_tensor(mbneg[:], pview(stm)[0:1],
                                               -1.0, rstd[:],
                                               op0=ALU.mult, op1=ALU.mult)
                pA = ps_o.tile([128, 2, 512], f32, tag="po", name="ln_A")
                pB = ps_o.tile([128, 2, 512], f32, tag="po", name="ln_B")
                for ci, (off, sz) in enumerate(CH):
                    nc.tensor.matmul(pA[:, ci, 0:sz], ones_row[0:1, :],
                                     rstd[0:1, off:off + sz], start=True, stop=True)
                    nc.tensor.matmul(pB[:, ci, 0:sz], ones_row[0:1, :],
                                     mbneg[0:1, off:off + sz], start=True, stop=True)
                for kt in range(4):
                    tmp = scr.tile([128, Cq], f32, tag="lntmp")
                    nc.vector.scalar_tensor_tensor(
                        tmp[:], x1T[:, kt, :], l2w[:, kt:kt + 1], pview(pA),
                        op0=ALU.mult, op1=ALU.mult)
                    nc.vector.scalar_tensor_tensor(
                        h2i[:, kt, :], pview(pB),
                        l2w[:, kt:kt + 1], tmp[:],
                        op0=ALU.mult, op1=ALU.add)

            # ---- mlp1 (+ mlp2 for nt 0,1 interleaved at lag 2) ----
            if phases >= 8:
                pm = {}

                def mlp2_emit(kp, nts):
                    for nt in nts:
                        for ci, (off, sz) in enumerate(CH):
                            nc.tensor.matmul(
                                pm[nt][:, ci, 0:sz],
                                w2[:, kp, :, 128 * nt:128 * (nt + 1)],
                                gT[:, 2 * kp:2 * kp + 2, off:off + sz],
                                perf_mode=mybir.MatmulPerfMode.DoubleRow,
                                start=(kp == 0), stop=False)
                    if kp == 7:     # + b2*8 per output feature, closes group
                        for nt in nts:
                            for ci, (off, sz) in enumerate(CH):
                                nc.tensor.matmul(
                                    pm[nt][:, ci, 0:sz],
                                    onesb[0:1, 1024 + 128 * nt:1152 + 128 * nt],
                                    onesb[0:1, off:off + sz],
                                    start=False, stop=True)

                def mlp2_fin(nt):
                    for ci, (off, sz) in enumerate(CH):
                        nc.vector.scalar_tensor_tensor(
                            yT[:, nt, off:off + sz], pm[nt][:, ci, 0:sz],
                            0.125, x1T[:, nt, off:off + sz],
                            op0=ALU.mult, op1=ALU.add)
                    nc.sync.dma_start(out_d[:, nt * Cq:(nt + 1) * Cq],
                                      yT[:, nt, :])

                for nt in range(16):
                    p = ps.tile([128, 2, 512], f32, tag="b2", name=f"m1{nt}")
                    for kp in range(2):
                        for ci, (off, sz) in enumerate(CH):
                            nc.tensor.matmul(
                                p[:, ci, 0:sz],
                                w1[:, kp, :, 128 * nt:128 * (nt + 1)],
                                h2i[:, 2 * kp:2 * kp + 2, off:off + sz],
                                perf_mode=mybir.MatmulPerfMode.DoubleRow,
                                start=(kp == 0), stop=(kp == 1))
                    nc.scalar.activation(gT[:, nt, :], pview(p),
                                         AF.Gelu, bias=b1[:, nt:nt + 1])
                    if phases >= 9:
                        if nt == 1:
                            pm[0] = ps_o.tile([128, 2, 512], f32, tag="po",
                                              name="pm0")
                            pm[1] = ps_o.tile([128, 2, 512], f32, tag="po",
                                              name="pm1")
                        if nt >= 3 and nt % 2 == 1:
                            mlp2_emit((nt - 3) // 2, (0, 1))
                if phases >= 9:
                    mlp2_emit(7, (0, 1))
                    mlp2_fin(0)
                    mlp2_fin(1)
                    pm[2] = ps.tile([128, 2, 512], f32, tag="b2", name="pm2")
                    pm[3] = ps.tile([128, 2, 512], f32, tag="b2", name="pm3")
                    for kp in range(8):
                        mlp2_emit(kp, (2, 3))
                    mlp2_fin(2)
                    mlp2_fin(3)

            if phases < 9:
                for nt in range(4):
                    nc.vector.tensor_copy(yT[:, nt, :], xT[:, nt, :])
                    nc.sync.dma_start(out_d[:, nt * Cq:(nt + 1) * Cq],
                                      yT[:, nt, :])

            att_po.__exit__(None, None, None)
            psA_cm.__exit__(None, None, None)

    _split_excess_waits(nc)
    return nc


_prog_cache = {}


def _get_prog(Cq, Ck):
    key = (Cq, Ck)
    if key not in _prog_cache:
        _prog_cache[key] = _build(Cq, Ck)
    return _prog_cache[key]


def _route(xf, gate_w, gate_b):
    """Replicate reference routing: top-2 of xf @ gate_w.T + gate_b."""
    logits = xf @ gate_w.T + gate_b            # [T, NE] fp32
    n = len(logits)
    idx0 = np.argmax(logits, axis=1)
    v0 = logits[np.arange(n), idx0]
    masked = logits.copy()
    masked[np.arange(n), idx0] = -np.inf
    idx1 = np.argmax(masked, axis=1)
    v1 = masked[np.arange(n), idx1]
    m = np.maximum(v0, v1)
    e0 = np.exp(v0 - m)
    e1 = np.exp(v1 - m)
    p0 = e0 / (e0 + e1)
    p1 = e1 / (e0 + e1)
    return np.stack([idx0, idx1], 1), np.stack([p0, p1], 1).astype(np.float32)


def _feat_pack(a, tiles, dtype):
    """[tiles*128, N] -> [128, tiles*N] with [p, t*N+c] = a[t*128+p, c]."""
    n = a.shape[1]
    return np.ascontiguousarray(
        a.reshape(tiles, 128, n).transpose(1, 0, 2).reshape(128, tiles * n)
        .astype(dtype))


def kernel(x, gate_w, gate_b, ln1_w, ln1_b, in_proj_w, in_proj_b, out_proj_w,
           out_proj_b, ln2_w, ln2_b, mlp_w1, mlp_b1, mlp_w2, mlp_b2):
    x = np.asarray(x, np.float32)
    B, N, _ = x.shape
    T = B * N
    xf = np.ascontiguousarray(x.reshape(T, E))

    topk_idx, probs = _route(xf, np.asarray(gate_w, np.float32),
                             np.asarray(gate_b, np.float32))

    groups = []          # (token_indices, prob_slice) per core, kk-major
    for kk in range(TOPK):
        for e in range(NE):
            sel = np.nonzero(topk_idx[:, kk] == e)[0]
            groups.append((sel, probs[sel, kk]))
    Cmax = max((len(s) for s, _ in groups), default=128)
    Cq = max(128, -(-Cmax // 64) * 64)
    Ck = max(128, -(-Cmax // 128) * 128)
    KT = Ck // 128

    bfd = ml_dtypes.bfloat16
    ew = []
    for e in range(NE):
        Wq = np.asarray(in_proj_w[e][0:E], np.float32)
        Wk = np.asarray(in_proj_w[e][E:2 * E], np.float32)
        Wv = np.asarray(in_proj_w[e][2 * E:3 * E], np.float32)
        bq = np.asarray(in_proj_b[e][0:E], np.float32)
        bk = np.asarray(in_proj_b[e][E:2 * E], np.float32)
        bv = np.asarray(in_proj_b[e][2 * E:3 * E], np.float32)
        l1b = np.asarray(ln1_b[e], np.float32)
        l2b = np.asarray(ln2_b[e], np.float32)
        scale = np.float32(1.0) / np.sqrt(np.float32(D))
        wqk = np.concatenate([Wq.T * scale, Wk.T], axis=1)          # [E, 2E]
        bqk = np.concatenate([(Wq @ l1b + bq) * scale, Wk @ l1b + bk])
        # pair-major slots: j=2t -> q pair t (nt=t), j=2t+1 -> k pair t (nt=4+t)
        arr = wqk.reshape(4, 128, 8, 128)          # [kt, p, nt, c]
        pm = arr[:, :, [0, 4, 1, 5, 2, 6, 3, 7], :]
        pm = pm.transpose(1, 2, 0, 3).reshape(128, 8, 512)
        w1 = np.asarray(mlp_w1[e], np.float32)
        fp8d = ml_dtypes.float8_e4m3
        # DoubleRow pair-major [p, kp, j, m] = W.T[(2kp+j)*128+p, m] * 8
        w1t8 = (w1.T * 8.0).reshape(2, 2, 128, HID).transpose(2, 0, 1, 3)
        w2t8 = (np.asarray(mlp_w2[e], np.float32).T * 8.0)\
            .reshape(8, 2, 128, E).transpose(2, 0, 1, 3)
        ew.append(dict(
            wqka=np.ascontiguousarray(pm[:, 0:2, :].reshape(128, 1024)
                                      .astype(bfd)),
            wqkb=np.ascontiguousarray(pm[:, 2:8, :].reshape(128, 3072)
                                      .astype(bfd)),
            bqk=np.ascontiguousarray(bqk, np.float32),
            wv=_feat_pack(Wv.T, 4, bfd),
            bvr=np.ascontiguousarray(Wv @ l1b + bv, np.float32),
            wo=_feat_pack(np.asarray(out_proj_w[e], np.float32).T, 4, bfd),
            bo=np.ascontiguousarray(out_proj_b[e], np.float32),
            w1=np.ascontiguousarray(w1t8.reshape(128, 8192).astype(fp8d)),
            b1=np.ascontiguousarray(w1 @ l2b + np.asarray(mlp_b1[e], np.float32)),
            w2=np.ascontiguousarray(w2t8.reshape(128, 8192).astype(fp8d)),
            b2=np.ascontiguousarray(mlp_b2[e], np.float32),
            l1w=np.ascontiguousarray(ln1_w[e], np.float32),
            l2w=np.ascontiguousarray(ln2_w[e], np.float32),
        ))

    def colpack(vec, ncol):
        a = np.zeros((128, ncol), np.float32)
        v = np.asarray(vec, np.float32).reshape(-1)
        a[:, :] = v.reshape(ncol, 128).T
        return a

    # rp broadcast selector: head h's reciprocal denominator lives on den
    # partition 32*(h%4), slot h//4; rp[t] (head pair 2t,2t+1) maps it to
    # output partitions 0:64 / 64:128. Block A: pairs on partitions {0,32},
    # block B: pairs on {64,96}.
    sel8_np = np.zeros((128, 256), np.float32)
    sel8_np[0, 0:64] = 1.0
    sel8_np[32, 64:128] = 1.0
    sel8_np[64, 128:192] = 1.0
    sel8_np[96, 192:256] = 1.0

    in_maps = []
    for ci, (sel, _p) in enumerate(groups):
        e = ci % NE
        w = ew[e]
        S = len(sel)
        xg = xf[sel]
        xgT = np.zeros((E, Cq), np.float32)
        xgT[:, :S] = xg.T
        mu_h = xg.mean(1, keepdims=True)
        var_h = ((xg - mu_h) ** 2).mean(1, keepdims=True)
        hg = (xg - mu_h) / np.sqrt(var_h + EPS) * w["l1w"][None, :]
        hT_np = np.zeros((E, Ck), np.float32)
        hT_np[:, :S] = hg.T
        kbias = np.full((Ck,), KEY_PAD_BIAS, np.float32)
        kbias[:S] = 0.0
        consts = np.concatenate([
            colpack(kbias, KT), colpack(w["bqk"], 8), colpack(w["bo"], 4),
            colpack(w["b1"], 16), colpack(w["b2"], 4),
            colpack(w["l2w"] / 8.0, 4), np.ones((128, 1), np.float32)], axis=1)
        onesb_np = np.zeros((1, 1536), np.float32)
        onesb_np[0, 0:512] = 1.0
        onesb_np[0, 512:1024] = w["bvr"]
        onesb_np[0, 1024:1536] = w["b2"] * 8.0
        in_maps.append({
            "hT": _feat_pack(hT_np, 4, bfd),
            "xgT": _feat_pack(xgT, 4, np.float32),
            "consts": consts,
            "sel8": sel8_np,
            "onesb": onesb_np.astype(bfd),
            "wqka": w["wqka"], "wqkb": w["wqkb"], "wv": w["wv"], "wo": w["wo"],
            "w1": w["w1"], "w2": w["w2"],
        })

    nc = _get_prog(Cq, Ck)
    res = bass_utils.run_bass_kernel_spmd(
        nc, in_maps, core_ids=list(range(8)),
        trace=bool(int(os.environ.get("KERNEL_TRACE", "0"))))
    kernel.last_exec_time_ns = res.exec_time_ns
    kernel.last_results = res

    out = np.zeros((T, E), np.float32)
    for ci, (sel, p) in enumerate(groups):
        S = len(sel)
        if S == 0:
            continue
        yT = np.asarray(res.results[ci]["yT"], np.float32)     # [128, 4*Cq]
        yTf = yT.reshape(128, 4, Cq).transpose(1, 0, 2).reshape(E, Cq)
        out[sel] += yTf[:, :S].T * p[:, None]
    return out.reshape(B, N, E)
